# revision 6
# baseline (speedup 1.0000x reference)
"""GRU (JitGRU_Predictor) Trainium2 Bass kernel.

Strategy (hardcoded for B=256, T=512, F=8, H=512, OUT=96, 8 cores):
  - Data parallel: batch 256 -> 8 cores x 32.
  - Per core, everything lives in a transposed layout [feature, batch] so the
    recurrent matmul h_res^T = W_hh^T.T @ h^T keeps features on partitions and
    the elementwise gate math runs on all 128 lanes.
  - W_hh^T is stationary-tiled (48 [128,128] bf16 tiles, FWL fast weight
    load); h^T [128, 4x32] bf16 is the moving operand; accumulation fp32 in
    PSUM. Three separate PSUM banks (r/z/n groups) per step so gate math can
    start before all 48 matmuls finish.
  - Input projections x@W_ih^T + biases are precomputed per 16-step chunk
    with K=9 matmuls (8 features + a ones-row carrying the combined bias),
    interleaved with the recurrence to fill PE bubbles.
  - Gates: zc = sigmoid(-pre_z) = 1-z on ScalarE (free negation via scale),
    h_new = (h - zc*h) + zc*n. fp32 pre-activations, bf16 gate outputs/state.
  - FC tail in fp32, output written transposed [96, 32]; host de-transposes.
"""

import os
import sys

import numpy as np

for _p in ("/opt/trn_rl_repo", "/root/.axon_site/_ro/trn_rl_repo"):
    if os.path.isdir(_p) and _p not in sys.path:
        sys.path.insert(0, _p)
        break

import ml_dtypes

import concourse.bass as bass
import concourse.mybir as mybir
from concourse import bacc
from concourse.bass_utils import run_bass_kernel_spmd
from concourse.tile import TileContext

BF16 = ml_dtypes.bfloat16

B, T, F, H, OUT = 256, 512, 8, 512, 96
NCORES = 8
BL = B // NCORES          # 32 batch rows per core
KC = H // 128             # 4 contraction chunks
MC = (3 * H) // 128       # 12 output-feature chunks
TCH = 16                  # timesteps per x-precompute chunk
NCHUNK = T // TCH

_CACHE = {}


def _build(t_steps=T):
    nchunk = max(1, t_steps // TCH)
    nc = bacc.Bacc("TRN2", target_bir_lowering=False, debug=False)
    dt = mybir.dt
    f32, bf16 = dt.float32, dt.bfloat16
    AF = mybir.ActivationFunctionType
    ds = bass.ds

    xT9 = nc.dram_tensor("xT9", [9, t_steps * BL], bf16, kind="ExternalInput")
    whhT = nc.dram_tensor("whhT", [H, 3 * H], bf16, kind="ExternalInput")
    wih9 = nc.dram_tensor("wih9", [9, 3 * H], bf16, kind="ExternalInput")
    fcwT = nc.dram_tensor("fcwT", [H, OUT], f32, kind="ExternalInput")
    fcb = nc.dram_tensor("fcb", [OUT, 1], f32, kind="ExternalInput")
    outT = nc.dram_tensor("outT", [OUT, BL], f32, kind="ExternalOutput")

    with TileContext(nc) as tc:
        with (
            tc.tile_pool(name="persist", bufs=1) as pp,
            tc.tile_pool(name="work", bufs=3) as wp,
            tc.tile_pool(name="hpool", bufs=2) as hp,
            tc.tile_pool(name="psum2", bufs=2, space="PSUM") as ps2,
            tc.tile_pool(name="psum1", bufs=1, space="PSUM") as ps1,
        ):
            whh_sb = pp.tile([128, KC, 3 * H], bf16, tag="whh")
            for c in range(KC):
                nc.sync.dma_start(out=whh_sb[:, c, :], in_=whhT[c * 128 : (c + 1) * 128, :])
            wih_sb = pp.tile([9, 3 * H], bf16, tag="wih")
            nc.sync.dma_start(out=wih_sb[:, :], in_=wih9[:, :])
            x_sb = pp.tile([9, t_steps * BL], bf16, tag="x")
            nc.sync.dma_start(out=x_sb[:, :], in_=xT9[:, :])
            fcw_sb = pp.tile([128, KC, OUT], f32, tag="fcw")
            for c in range(KC):
                nc.sync.dma_start(out=fcw_sb[:, c, :], in_=fcwT[c * 128 : (c + 1) * 128, :])
            fcb_sb = pp.tile([OUT, 1], f32, tag="fcb")
            nc.sync.dma_start(out=fcb_sb[:, :], in_=fcb[:, :])

            # step-major layout: per-step slices are contiguous 2D APs
            # (3D strided APs overflow the ISA sync-slot budget in walrus).
            xr = [
                pp.tile([128, TCH, MC * BL], f32, tag=f"xr{i}", name=f"xr{i}")
                for i in range(2)
            ]

            def precompute(chunk, jlist):
                par = chunk % 2
                for j in jlist:
                    psx = ps1.tile([128, TCH * BL], f32, tag="psx")
                    nc.tensor.matmul(
                        psx,
                        wih_sb[:, ds(j * 128, 128)],
                        x_sb[:, ds(chunk * TCH * BL, TCH * BL)],
                        start=True,
                        stop=True,
                    )
                    dst = xr[par][:, :, j * BL : (j + 1) * BL]
                    srcv = psx[:, :].rearrange("p (s b) -> p s b", b=BL)
                    if j % 2 == 0:
                        nc.scalar.copy(dst, srcv)
                    else:
                        nc.vector.tensor_copy(dst, srcv)

            htb = hp.tile([128, KC, BL], bf16, tag="htb")
            nc.vector.memset(htb, 0.0)
            precompute(0, range(MC))

            for t in range(t_steps):
                chunk, s = divmod(t, TCH)
                par = chunk % 2
                ps_z = ps2.tile([128, 4, BL], f32, tag="psz")
                ps_r = ps2.tile([128, 4, BL], f32, tag="psr")
                ps_n = ps2.tile([128, 4, BL], f32, tag="psn")
                zc = rr = m1 = pre = None
                # z group first, then r, then n: lets sigmoid(z)/sigmoid(r)
                # overlap the remaining matmuls (separate PSUM banks).
                for grp, pst in ((1, ps_z), (0, ps_r), (2, ps_n)):
                    for jj in range(4):
                        j = grp * 4 + jj
                        for c in range(KC):
                            nc.tensor.matmul(
                                pst[:, jj, :],
                                whh_sb[:, c, ds(j * 128, 128)],
                                htb[:, c, :],
                                start=(c == 0),
                                stop=(c == KC - 1),
                            )
                    if grp == 1:
                        nc.vector.tensor_add(ps_z, ps_z, xr[par][:, s, 4 * BL : 8 * BL])
                        zc = wp.tile([128, 4, BL], bf16, tag="zc")
                        nc.scalar.activation(zc, ps_z, AF.Sigmoid, scale=-1.0)
                    elif grp == 0:
                        nc.vector.tensor_add(ps_r, ps_r, xr[par][:, s, 0 : 4 * BL])
                        rr = wp.tile([128, 4, BL], f32, tag="rr")
                        nc.scalar.activation(rr, ps_r, AF.Sigmoid)
                        m1 = wp.tile([128, KC, BL], bf16, tag="m1")
                        nc.vector.tensor_mul(m1, zc, htb)
                        pre = wp.tile([128, KC, BL], bf16, tag="pre")
                        nc.vector.tensor_sub(pre, htb, m1)
                # x-projection for the next chunk: PE bubble filler.
                if s < MC and chunk + 1 < nchunk:
                    precompute(chunk + 1, [s])
                htb_new = hp.tile([128, KC, BL], bf16, tag="htb")
                t1 = wp.tile([128, 4, BL], f32, tag="t1")
                nn = wp.tile([128, 4, BL], bf16, tag="nn")
                for hh in range(2):
                    sl = slice(2 * hh, 2 * hh + 2)
                    nc.vector.tensor_mul(t1[:, sl, :], rr[:, sl, :], ps_n[:, sl, :])
                    nc.vector.tensor_add(
                        t1[:, sl, :],
                        t1[:, sl, :],
                        xr[par][:, s, (8 + 2 * hh) * BL : (10 + 2 * hh) * BL],
                    )
                    nc.scalar.activation(nn[:, sl, :], t1[:, sl, :], AF.Tanh)
                    nc.vector.tensor_mul(nn[:, sl, :], zc[:, sl, :], nn[:, sl, :])
                    nc.vector.tensor_add(htb_new[:, sl, :], pre[:, sl, :], nn[:, sl, :])
                htb = htb_new

            hT = wp.tile([128, KC, BL], f32, tag="hT")
            nc.vector.tensor_copy(hT, htb)
            psf = ps1.tile([OUT, BL], f32, tag="psf")
            for c in range(KC):
                nc.tensor.matmul(
                    psf, fcw_sb[:, c, :], hT[:, c, :], start=(c == 0), stop=(c == KC - 1)
                )
            ot = wp.tile([OUT, BL], f32, tag="ot")
            nc.vector.tensor_scalar_add(ot, psf, fcb_sb)
            nc.sync.dma_start(out=outT[:, :], in_=ot[:, :])
    nc.compile()
    return nc


def _get_nc(t_steps=T):
    if t_steps not in _CACHE:
        _CACHE[t_steps] = _build(t_steps)
    return _CACHE[t_steps]


def _in_maps(x, weight_ih, weight_hh, bias_ih, bias_hh, fc_w, fc_b, t_steps=T):
    whhT = np.ascontiguousarray(weight_hh.astype(np.float32).T).astype(BF16)
    wih9 = np.concatenate(
        [weight_ih.astype(np.float32).T, (bias_ih + bias_hh).astype(np.float32)[None, :]], 0
    ).astype(BF16)
    fcwT = np.ascontiguousarray(fc_w.astype(np.float32).T)
    fcbc = np.ascontiguousarray(fc_b.astype(np.float32).reshape(OUT, 1))
    maps = []
    for i in range(NCORES):
        xs = np.asarray(x[i * BL : (i + 1) * BL, :t_steps], np.float32)  # [BL, t, F]
        xT = np.ascontiguousarray(xs.transpose(2, 1, 0)).reshape(F, t_steps * BL)
        xT9 = np.concatenate([xT, np.ones((1, t_steps * BL), np.float32)], 0).astype(BF16)
        maps.append({"xT9": xT9, "whhT": whhT, "wih9": wih9, "fcwT": fcwT, "fcb": fcbc})
    return maps


def run(x, weight_ih, weight_hh, bias_ih, bias_hh, fc_w, fc_b, t_steps=T, **spmd_kwargs):
    nc = _get_nc(t_steps)
    maps = _in_maps(x, weight_ih, weight_hh, bias_ih, bias_hh, fc_w, fc_b, t_steps)
    res = run_bass_kernel_spmd(nc, maps, list(range(NCORES)), **spmd_kwargs)
    out = np.concatenate([np.asarray(res.results[i]["outT"]).T for i in range(NCORES)], 0)
    return np.ascontiguousarray(out.astype(np.float32)), res


def kernel(x, weight_ih, weight_hh, bias_ih, bias_hh, fc_w, fc_b):
    out, _ = run(x, weight_ih, weight_hh, bias_ih, bias_hh, fc_w, fc_b)
    return out
